# revision 17
# baseline (speedup 1.0000x reference)
"""BiMamba (bidirectional Mamba-1 block) Trainium2 kernel.

Problem: B=2, L=1024, d_model=768, d_inner=1536, d_state=16, dt_rank=48,
d_conv=4; two directions (fwd on x, rev on flip(x)) sharing in/out
projections, outputs added.

Sharding over 8 NeuronCores: core = (direction, quarter of d_inner).
Cores 0-3 forward, 4-7 reverse (host flips x along L for them and unflips
their partial outputs).  Each core owns 384 d_inner channels = 3 partition
blocks of 128.

On-device layout: channels on partitions, time on the free dimension
(t = b*1024 + l, batches concatenated).  Per core:
  - in_proj (u and z slices), depthwise conv (4 accumulating diagonal
    matmuls), x_proj partial, dt_proj, out_proj partial: TensorE, bf16.
  - partial x_proj outputs ([80, 1024] per batch) AllReduce'd across the
    4 cores of the same direction.
  - softplus / exp(A*delta) (per-partition scale) / silu: ScalarE.
  - selective scan: chained tensor_tensor_scan on VectorE (scan opcode
    does not exist on GpSimd); dBu / y*C bf16 tensor_tensor passes split
    between VectorE and GpSimd.
  - sum over d_state: 16 identity matmuls accumulating in PSUM (TensorE).
  - out_proj partials DMA'd PSUM->DRAM directly; host sums the 8 partial
    (768, 2048) outputs and undoes transpose/flip.
"""
import sys

sys.path.insert(0, "/opt/trn_rl_repo")

import numpy as np
import ml_dtypes

import concourse.bass as bass
import concourse.bacc as bacc
import concourse.mybir as mybir
import concourse.tile as tile
from concourse.bass_utils import run_bass_kernel_spmd

# ---- problem constants ----
B, L, DM, E, S, R, K = 2, 1024, 768, 1536, 16, 48, 4
NCORES = 8
Q = 4                      # d_inner quarters per direction
DSL = E // Q               # 384 channels per core
NDB = DSL // 128           # 3 partition blocks
T = B * L                  # 2048 tokens, b-major
TC = 512                   # time chunk
NTC = L // TC              # chunks per batch
SG = 2                     # d_state groups
SPG = S // SG              # 8 states per group
NR = R + 2 * S             # 80
NKC = DM // 128            # 6 k-chunks of d_model

bf16 = mybir.dt.bfloat16
f32 = mybir.dt.float32
MUL = mybir.AluOpType.mult
ADD = mybir.AluOpType.add
AF = mybir.ActivationFunctionType

_NC_CACHE = None


def _bcast_free(ap_row, n):
    """Broadcast an AP of shape [p, F] to [p, n, F] with step-0 middle dim."""
    return bass.AP(tensor=ap_row.tensor, offset=ap_row.offset,
                   ap=[ap_row.ap[0], [0, n], ap_row.ap[1]])


def build_nc():
    nc = bacc.Bacc("TRN2", target_bir_lowering=False, debug=False,
                   num_devices=NCORES)

    # ---------------- I/O ----------------
    xT = nc.dram_tensor("xT", [DM, T], bf16, kind="ExternalInput")
    wu = nc.dram_tensor("wu", [DM, DSL], bf16, kind="ExternalInput")
    wz = nc.dram_tensor("wz", [DM, DSL], bf16, kind="ExternalInput")
    wcd = nc.dram_tensor("wcd", [NDB, K, 128, 128], bf16, kind="ExternalInput")
    wxp = nc.dram_tensor("wxp", [NDB, 128, NR], bf16, kind="ExternalInput")
    wdt = nc.dram_tensor("wdt", [R, DSL], bf16, kind="ExternalInput")
    wo = nc.dram_tensor("wo", [NDB, 128, DM], bf16, kind="ExternalInput")
    dtb = nc.dram_tensor("dtb", [NDB, 128, 1], f32, kind="ExternalInput")
    Aneg = nc.dram_tensor("Aneg", [NDB, 128, S], f32, kind="ExternalInput")
    Dp = nc.dram_tensor("Dp", [NDB, 128, 1], f32, kind="ExternalInput")
    cb = nc.dram_tensor("cb", [NDB, 128, 1], f32, kind="ExternalInput")
    ident = nc.dram_tensor("ident", [128, 128], bf16, kind="ExternalInput")
    opart = nc.dram_tensor("opart", [DM, T], f32, kind="ExternalOutput")

    with tile.TileContext(nc) as tc:
        import contextlib
        ctx = contextlib.ExitStack()
        with ctx:
            ctx.enter_context(nc.allow_low_precision(reason="deliberate bf16 pipeline"))
            singles = ctx.enter_context(tc.tile_pool(name="singles", bufs=1))
            xs_pool = ctx.enter_context(tc.tile_pool(name="xs", bufs=4))
            big = ctx.enter_context(tc.tile_pool(name="big", bufs=2))
            hpool = ctx.enter_context(tc.tile_pool(name="hp", bufs=2))
            bcpool = ctx.enter_context(tc.tile_pool(name="bc", bufs=2))
            small = ctx.enter_context(tc.tile_pool(name="small", bufs=2))
            actp = ctx.enter_context(tc.tile_pool(name="actp", bufs=2))
            redp = ctx.enter_context(tc.tile_pool(name="redp", bufs=1))
            y2pool = ctx.enter_context(tc.tile_pool(name="y2", bufs=4))
            psum = ctx.enter_context(tc.tile_pool(name="psum", bufs=6, space="PSUM"))
            dram = ctx.enter_context(tc.tile_pool(name="dram", bufs=1, space="DRAM"))

            # ------------- persistent weights -------------
            wu_sb = singles.tile([128, NKC, DSL], bf16)   # [k, kc, m]
            nc.sync.dma_start(out=wu_sb, in_=wu.ap().rearrange("(kc k) m -> k kc m", k=128))
            wz_sb = singles.tile([128, NKC, DSL], bf16)
            nc.sync.dma_start(out=wz_sb, in_=wz.ap().rearrange("(kc k) m -> k kc m", k=128))
            wcd_sb = singles.tile([128, NDB, K, 128], bf16)
            nc.sync.dma_start(out=wcd_sb, in_=wcd.ap().rearrange("db k p m -> p db k m"))
            wxp_sb = singles.tile([128, NDB, NR], bf16)
            nc.sync.dma_start(out=wxp_sb, in_=wxp.ap().rearrange("db p m -> p db m"))
            wdt_sb = singles.tile([R, DSL], bf16)
            nc.sync.dma_start(out=wdt_sb, in_=wdt.ap())
            wo_sb = singles.tile([128, NDB, DM], bf16)
            nc.sync.dma_start(out=wo_sb, in_=wo.ap().rearrange("db p m -> p db m"))
            dtb_sb = singles.tile([128, NDB, 1], f32)
            nc.sync.dma_start(out=dtb_sb, in_=dtb.ap().rearrange("db p o -> p db o"))
            A_sb = singles.tile([128, NDB, S], f32)
            nc.sync.dma_start(out=A_sb, in_=Aneg.ap().rearrange("db p s -> p db s"))
            Dp_sb = singles.tile([128, NDB, 1], f32)
            nc.sync.dma_start(out=Dp_sb, in_=Dp.ap().rearrange("db p o -> p db o"))
            cb_sb = singles.tile([128, NDB, 1], f32)
            nc.sync.dma_start(out=cb_sb, in_=cb.ap().rearrange("db p o -> p db o"))
            cbn_sb = singles.tile([128, NDB, 1], f32)
            nc.vector.tensor_scalar_mul(cbn_sb, cb_sb, -1.0)
            id_sb = singles.tile([128, 128], bf16)
            nc.sync.dma_start(out=id_sb, in_=ident.ap())

            # persistent activations
            u_sb = singles.tile([128, NDB, B, 3 + L], bf16)  # conv input, 3-pad
            uc_sb = singles.tile([128, NDB, T], bf16)        # silu(conv(u))
            sz_sb = singles.tile([128, NDB, T], bf16)        # silu(z)
            dtT_sb = singles.tile([R, T], bf16)              # reduced dt^T
            bcT_sb = singles.tile([2 * S, T], bf16)          # reduced B/C rows
            carry = singles.tile([128, NDB, S], f32)         # scan carries

            for db in range(NDB):
                for b in range(B):
                    nc.vector.memset(u_sb[:, db, b, 0:3], 0.0)

            # ------------- phase 1: in_proj u -------------
            for b in range(B):
                for tcn in range(NTC):
                    t0 = b * L + tcn * TC
                    for db in range(NDB):
                        pu = psum.tile([128, TC], f32, tag="mm")
                        for kc in range(NKC):
                            xs = xs_pool.tile([128, TC], bf16, tag="xs")
                            nc.sync.dma_start(
                                out=xs, in_=xT.ap()[kc * 128:(kc + 1) * 128, t0:t0 + TC])
                            nc.tensor.matmul(
                                pu, wu_sb[:, kc, db * 128:(db + 1) * 128], xs,
                                start=(kc == 0), stop=(kc == NKC - 1))
                        nc.scalar.copy(
                            u_sb[:, db, b, 3 + tcn * TC: 3 + (tcn + 1) * TC], pu)

            # ------------- phase 2: conv + silu -------------
            for b in range(B):
                for tcn in range(NTC):
                    t0 = b * L + tcn * TC
                    for db in range(NDB):
                        pc = psum.tile([128, TC], f32, tag="mm")
                        for k in range(K):
                            nc.tensor.matmul(
                                pc, wcd_sb[:, db, k, :],
                                u_sb[:, db, b, tcn * TC + k: tcn * TC + k + TC],
                                start=(k == 0), stop=(k == K - 1))
                        # silu(pc+cb) = x * 1/(1+exp(-x)); only Exp/Ln/Copy/
                        # Identity live in one ACT LUT table, so build it.
                        xsl = actp.tile([128, TC], bf16, tag="xsl")
                        nc.scalar.activation(xsl, pc, AF.Identity,
                                             bias=cb_sb[:, db, :], scale=1.0)
                        esl = actp.tile([128, TC], bf16, tag="esl")
                        nc.scalar.activation(esl, pc, AF.Exp,
                                             bias=cbn_sb[:, db, :], scale=-1.0)
                        dsl = actp.tile([128, TC], bf16, tag="dsl")
                        nc.gpsimd.tensor_scalar_add(dsl, esl, 1.0)
                        rsl = actp.tile([128, TC], bf16, tag="rsl")
                        nc.vector.reciprocal(rsl, dsl)
                        nc.gpsimd.tensor_tensor(uc_sb[:, db, t0:t0 + TC], xsl, rsl, MUL)

            # ------------- phase 3+4: x_proj partial -> allreduce ----------
            bcst = dram.tile([2 * S, T], bf16)       # broadcast staging
            for b in range(B):
                cin = dram.tile([NR, L], f32, tag="cin")
                cout = dram.tile([NR, L], f32, tag="cout")
                for tcn in range(NTC):
                    t0 = b * L + tcn * TC
                    px = psum.tile([NR, TC], f32, tag="mm")
                    for db in range(NDB):
                        nc.tensor.matmul(px, wxp_sb[:, db, :], uc_sb[:, db, t0:t0 + TC],
                                         start=(db == 0), stop=(db == NDB - 1))
                    xps = redp.tile([NR, TC], f32, tag="xps")
                    nc.scalar.copy(xps, px)
                    nc.sync.dma_start(out=cin[:, tcn * TC:(tcn + 1) * TC], in_=xps)
                nc.gpsimd.collective_compute(
                    "AllReduce", ADD,
                    replica_groups=[[0, 1, 2, 3], [4, 5, 6, 7]],
                    ins=[cin.opt()], outs=[cout.opt()],
                )
                red = redp.tile([R, L], f32, tag="red")
                redbc = redp.tile([2 * S, L], f32, tag="redbc")
                nc.sync.dma_start(out=red, in_=cout[0:R, :])
                nc.sync.dma_start(out=redbc, in_=cout[R:NR, :])
                nc.vector.tensor_copy(dtT_sb[:, b * L:(b + 1) * L], red)
                nc.vector.tensor_copy(bcT_sb[:, b * L:(b + 1) * L], redbc)
                nc.sync.dma_start(out=bcst[:, b * L:(b + 1) * L],
                                  in_=bcT_sb[:, b * L:(b + 1) * L])

            # ------------- phase 5: in_proj z + silu (overlaps collective) --
            for b in range(B):
                for tcn in range(NTC):
                    t0 = b * L + tcn * TC
                    for db in range(NDB):
                        pz = psum.tile([128, TC], f32, tag="mm")
                        for kc in range(NKC):
                            xs = xs_pool.tile([128, TC], bf16, tag="xs")
                            nc.sync.dma_start(
                                out=xs, in_=xT.ap()[kc * 128:(kc + 1) * 128, t0:t0 + TC])
                            nc.tensor.matmul(
                                pz, wz_sb[:, kc, db * 128:(db + 1) * 128], xs,
                                start=(kc == 0), stop=(kc == NKC - 1))
                        xsl = actp.tile([128, TC], bf16, tag="xsl")
                        nc.scalar.copy(xsl, pz)
                        esl = actp.tile([128, TC], bf16, tag="esl")
                        nc.scalar.activation(esl, pz, AF.Exp, scale=-1.0)
                        dsl = actp.tile([128, TC], bf16, tag="dsl")
                        nc.gpsimd.tensor_scalar_add(dsl, esl, 1.0)
                        rsl = actp.tile([128, TC], bf16, tag="rsl")
                        nc.vector.reciprocal(rsl, dsl)
                        nc.gpsimd.tensor_tensor(sz_sb[:, db, t0:t0 + TC], xsl, rsl, MUL)

            # ------------- phases 6-8: heavy per-(b, tc) pipeline -----------
            for b in range(B):
                for tcn in range(NTC):
                    t0 = b * L + tcn * TC
                    # broadcast B/C rows for this chunk (per s-group tiles)
                    bbc = []
                    cbc = []
                    for sg in range(SG):
                        bt = bcpool.tile([128, SPG, TC], bf16, tag="bbc")
                        ct = bcpool.tile([128, SPG, TC], bf16, tag="cbc")
                        for i in range(SPG):
                            s = sg * SPG + i
                            rb = bcst[s:s + 1, t0:t0 + TC]
                            rc = bcst[S + s:S + s + 1, t0:t0 + TC]
                            nc.sync.dma_start(
                                out=bt[:, i, :],
                                in_=bass.AP(tensor=rb.tensor, offset=rb.offset,
                                            ap=[[0, 128], rb.ap[-1]]))
                            nc.sync.dma_start(
                                out=ct[:, i, :],
                                in_=bass.AP(tensor=rc.tensor, offset=rc.offset,
                                            ap=[[0, 128], rc.ap[-1]]))
                        bbc.append(bt)
                        cbc.append(ct)

                    ys = []
                    for db in range(NDB):
                        # dt_proj -> softplus -> delta (bf16)
                        pdt = psum.tile([128, TC], f32, tag="mm")
                        nc.tensor.matmul(pdt, wdt_sb[:, db * 128:(db + 1) * 128],
                                         dtT_sb[:, t0:t0 + TC], start=True, stop=True)
                        # softplus(pdt + dtb) = Ln(1 + Exp(pdt + dtb))
                        e1 = actp.tile([128, TC], f32, tag="e1")
                        nc.scalar.activation(e1, pdt, AF.Exp,
                                             bias=dtb_sb[:, db, :], scale=1.0)
                        delta = small.tile([128, TC], bf16, tag="delta")
                        nc.scalar.activation(delta, e1, AF.Ln, bias=1.0, scale=1.0)
                        # w = delta * uc
                        w = small.tile([128, TC], bf16, tag="w")
                        nc.vector.tensor_tensor(w, delta, uc_sb[:, db, t0:t0 + TC], MUL)

                        py = psum.tile([128, TC], f32, tag="mm")
                        for sg in range(SG):
                            da = big.tile([128, SPG, TC], bf16, tag="da")
                            for i in range(SPG):
                                s = sg * SPG + i
                                nc.scalar.activation(da[:, i, :], delta, AF.Exp,
                                                     scale=A_sb[:, db, s:s + 1])
                            dbu = big.tile([128, SPG, TC], bf16, tag="dbu")
                            eng = nc.vector if sg == 0 else nc.gpsimd
                            eng.tensor_tensor(dbu, _bcast_free(w, SPG),
                                              bbc[sg], MUL)
                            h = hpool.tile([128, SPG, TC], bf16, tag="h")
                            for i in range(SPG):
                                s = sg * SPG + i
                                init = 0.0 if tcn == 0 else carry[:, db, s:s + 1]
                                nc.vector.tensor_tensor_scan(
                                    h[:, i, :], da[:, i, :], dbu[:, i, :], init,
                                    MUL, ADD)
                            if tcn + 1 < NTC:
                                nc.vector.tensor_copy(
                                    carry[:, db, sg * SPG:(sg + 1) * SPG],
                                    h[:, :, TC - 1])
                            yp = big.tile([128, SPG, TC], bf16, tag="yp")
                            eng2 = nc.gpsimd if sg == 0 else nc.vector
                            eng2.tensor_tensor(yp, h, cbc[sg], MUL)
                            for i in range(SPG):
                                nc.tensor.matmul(py, id_sb, yp[:, i, :],
                                                 start=(sg == 0 and i == 0),
                                                 stop=(sg == SG - 1 and i == SPG - 1))
                        # y1 = uc*Dp + py ; y2 = y1 * silu(z)
                        y1 = small.tile([128, TC], bf16, tag="y1")
                        nc.vector.scalar_tensor_tensor(
                            y1, uc_sb[:, db, t0:t0 + TC], Dp_sb[:, db, :], py,
                            MUL, ADD)
                        y2t = y2pool.tile([128, TC], bf16, tag="y2")
                        nc.vector.tensor_tensor(y2t, y1, sz_sb[:, db, t0:t0 + TC], MUL)
                        ys.append(y2t)

                    # out_proj partial for this chunk
                    for mc in range(NKC):
                        po = psum.tile([128, TC], f32, tag="mm")
                        for db in range(NDB):
                            nc.tensor.matmul(
                                po, wo_sb[:, db, mc * 128:(mc + 1) * 128], ys[db],
                                start=(db == 0), stop=(db == NDB - 1))
                        ot = y2pool.tile([128, TC], f32, tag="ot")
                        nc.scalar.copy(ot, po)
                        nc.sync.dma_start(
                            out=opart.ap()[mc * 128:(mc + 1) * 128, t0:t0 + TC],
                            in_=ot)
    nc.compile()
    return nc


def _get_nc():
    global _NC_CACHE
    if _NC_CACHE is None:
        _NC_CACHE = build_nc()
    return _NC_CACHE


def _bf(a):
    return np.ascontiguousarray(a).astype(ml_dtypes.bfloat16)


def kernel(**inputs):
    hs = np.asarray(inputs["hidden_states"], dtype=np.float32)  # (B, L, DM)
    in_w = np.asarray(inputs["in_proj_w"], dtype=np.float32)    # (2E, DM)
    out_w = np.asarray(inputs["out_proj_w"], dtype=np.float32)  # (DM, E)
    ident = np.eye(128, dtype=np.float32)

    in_maps = []
    for c in range(NCORES):
        d = "f" if c < 4 else "r"
        q = c % 4
        sl = slice(q * DSL, (q + 1) * DSL)
        x = hs if d == "f" else hs[:, ::-1, :]
        xTh = np.ascontiguousarray(x.transpose(2, 0, 1).reshape(DM, T))

        cw = np.asarray(inputs[f"conv_w_{d}"], dtype=np.float32)[sl]   # (384, 4)
        wcdh = np.zeros((NDB, K, 128, 128), np.float32)
        for db in range(NDB):
            for k in range(K):
                np.fill_diagonal(wcdh[db, k], cw[db * 128:(db + 1) * 128, k])

        xpw = np.asarray(inputs[f"x_proj_w_{d}"], dtype=np.float32)    # (80, E)
        wxph = np.stack([xpw[:, q * DSL + db * 128: q * DSL + (db + 1) * 128].T
                         for db in range(NDB)])                         # (3,128,80)
        dtw = np.asarray(inputs[f"dt_w_{d}"], dtype=np.float32)[sl]    # (384, 48)
        woh = np.stack([out_w[:, q * DSL + db * 128: q * DSL + (db + 1) * 128].T
                        for db in range(NDB)])                          # (3,128,768)

        in_maps.append({
            "xT": _bf(xTh),
            "wu": _bf(in_w[sl].T),
            "wz": _bf(in_w[E:][sl].T),
            "wcd": _bf(wcdh),
            "wxp": _bf(wxph),
            "wdt": _bf(dtw.T),
            "wo": _bf(woh),
            "dtb": np.ascontiguousarray(
                np.asarray(inputs[f"dt_b_{d}"], np.float32)[sl]).reshape(NDB, 128, 1),
            "Aneg": np.ascontiguousarray(
                -np.exp(np.asarray(inputs[f"A_log_{d}"], np.float32)[sl])).reshape(NDB, 128, S),
            "Dp": np.ascontiguousarray(
                np.asarray(inputs[f"D_{d}"], np.float32)[sl]).reshape(NDB, 128, 1),
            "cb": np.ascontiguousarray(
                np.asarray(inputs[f"conv_b_{d}"], np.float32)[sl]).reshape(NDB, 128, 1),
            "ident": _bf(ident),
        })

    nc = _get_nc()
    res = run_bass_kernel_spmd(nc, in_maps, core_ids=list(range(NCORES)))

    acc_f = np.zeros((DM, T), np.float32)
    acc_r = np.zeros((DM, T), np.float32)
    for c in range(NCORES):
        if c < 4:
            acc_f += res.results[c]["opart"]
        else:
            acc_r += res.results[c]["opart"]
    out_f = acc_f.reshape(DM, B, L).transpose(1, 2, 0)
    out_r = acc_r.reshape(DM, B, L).transpose(1, 2, 0)[:, ::-1, :]
    return np.ascontiguousarray(out_f + out_r, dtype=np.float32)


# revision 43
# speedup vs baseline: 137.5708x; 137.5708x over previous
"""BiMamba (bidirectional Mamba-1 block) Trainium2 kernel.

Problem: B=2, L=1024, d_model=768, d_inner=1536, d_state=16, dt_rank=48,
d_conv=4; two directions (fwd on x, rev on flip(x)) sharing in/out
projections, outputs added.

Sharding over 8 NeuronCores: core = (direction, quarter of d_inner).
Cores 0-3 forward, 4-7 reverse (host flips x along L for them and unflips
their partial outputs).  Each core owns 384 d_inner channels = 3 partition
blocks of 128.

On-device layout: channels on partitions, time on the free dimension
(t = b*1024 + l, batches concatenated).  Per core:
  - in_proj (u and z slices), depthwise conv (4 accumulating diagonal
    matmuls), x_proj partial, dt_proj, out_proj partial: TensorE, bf16.
  - partial x_proj outputs ([80, 1024] per batch) AllReduce'd across the
    4 cores of the same direction.
  - softplus / exp(A*delta) (per-partition scale) / silu: ScalarE.
  - selective scan: chained tensor_tensor_scan on VectorE (scan opcode
    does not exist on GpSimd); dBu / y*C bf16 tensor_tensor passes split
    between VectorE and GpSimd.
  - sum over d_state: 16 identity matmuls accumulating in PSUM (TensorE).
  - out_proj partials copied PSUM->SBUF (ScalarE) and DMA'd out; the host
    sums the 8 partial (768, 2048) fp32 outputs and undoes transpose/flip.

TimelineSim (cost model, collective stubbed): ~310 us/core end-to-end;
engine busy: Pool 228 us, ACT 216 us, DVE 207 us, PE 137 us.
Measured end-to-end relative L2 error vs the fp32 reference: 6.9e-3.
"""
import sys

sys.path.insert(0, "/opt/trn_rl_repo")

import numpy as np
import ml_dtypes

import concourse.bass as bass
import concourse.bacc as bacc
import concourse.mybir as mybir
import concourse.tile as tile
from concourse.bass_utils import run_bass_kernel_spmd

# ---- problem constants ----
B, L, DM, E, S, R, K = 2, 1024, 768, 1536, 16, 48, 4
NCORES = 8
Q = 4                      # d_inner quarters per direction
DSL = E // Q               # 384 channels per core
NDB = DSL // 128           # 3 partition blocks
T = B * L                  # 2048 tokens, b-major
TC = 512                   # time chunk
NTC = L // TC              # chunks per batch
SG = 2                     # d_state groups
SPG = S // SG              # 8 states per group
NR = R + 2 * S             # 80
NKC = DM // 128            # 6 k-chunks of d_model

bf16 = mybir.dt.bfloat16
f32 = mybir.dt.float32
MUL = mybir.AluOpType.mult
ADD = mybir.AluOpType.add
AF = mybir.ActivationFunctionType

_NC_CACHE = None


def _bcast_free(ap_row, n):
    """Broadcast an AP of shape [p, F] to [p, n, F] with step-0 middle dim."""
    return bass.AP(tensor=ap_row.tensor, offset=ap_row.offset,
                   ap=[ap_row.ap[0], [0, n], ap_row.ap[1]])


def build_nc(sim_mode=False):
    """sim_mode=True: single-core, collective replaced by a DRAM->DRAM DMA
    (same dataflow deps) so TimelineSim can run the kernel."""
    nc = bacc.Bacc("TRN2", target_bir_lowering=False, debug=False,
                   num_devices=1 if sim_mode else NCORES)

    # ---------------- I/O ----------------
    xT = nc.dram_tensor("xT", [DM, T], bf16, kind="ExternalInput")
    wu = nc.dram_tensor("wu", [128, NKC, DSL], bf16, kind="ExternalInput")
    wz = nc.dram_tensor("wz", [128, NKC, DSL], bf16, kind="ExternalInput")
    wcd = nc.dram_tensor("wcd", [128, NDB, K, 128], bf16, kind="ExternalInput")
    wxp = nc.dram_tensor("wxp", [128, NDB, NR], bf16, kind="ExternalInput")
    wdt = nc.dram_tensor("wdt", [R, DSL], bf16, kind="ExternalInput")
    wo = nc.dram_tensor("wo", [128, NDB, DM], bf16, kind="ExternalInput")
    dtb = nc.dram_tensor("dtb", [128, NDB, 1], f32, kind="ExternalInput")
    Aneg = nc.dram_tensor("Aneg", [128, NDB, S], f32, kind="ExternalInput")
    Dp = nc.dram_tensor("Dp", [128, NDB, 1], f32, kind="ExternalInput")
    cb = nc.dram_tensor("cb", [128, NDB, 1], f32, kind="ExternalInput")
    ident = nc.dram_tensor("ident", [128, 128], bf16, kind="ExternalInput")
    opart = nc.dram_tensor("opart", [DM, T], f32, kind="ExternalOutput")

    with tile.TileContext(nc) as tc:
        import contextlib
        ctx = contextlib.ExitStack()
        with ctx:
            ctx.enter_context(nc.allow_low_precision(reason="deliberate bf16 pipeline"))
            singles = ctx.enter_context(tc.tile_pool(name="singles", bufs=1))
            xs_pool = ctx.enter_context(tc.tile_pool(name="xs", bufs=8))
            big = ctx.enter_context(tc.tile_pool(name="big", bufs=2))
            hpool = ctx.enter_context(tc.tile_pool(name="hp", bufs=3))
            bcpool = ctx.enter_context(tc.tile_pool(name="bc", bufs=2))
            small = ctx.enter_context(tc.tile_pool(name="small", bufs=3))
            actp = ctx.enter_context(tc.tile_pool(name="actp", bufs=2))
            redp = ctx.enter_context(tc.tile_pool(name="redp", bufs=1))
            y2pool = ctx.enter_context(tc.tile_pool(name="y2", bufs=4))
            otpool = ctx.enter_context(tc.tile_pool(name="otp", bufs=2))
            psum = ctx.enter_context(tc.tile_pool(name="psum", bufs=3, space="PSUM"))
            psumy = ctx.enter_context(tc.tile_pool(name="psumy", bufs=3, space="PSUM"))
            psumo = ctx.enter_context(tc.tile_pool(name="psumo", bufs=2, space="PSUM"))
            dram = ctx.enter_context(tc.tile_pool(name="dram", bufs=1, space="DRAM"))

            # ------------- persistent weights -------------
            wu_sb = singles.tile([128, NKC, DSL], bf16)   # [k, kc, m]
            nc.sync.dma_start(out=wu_sb, in_=wu.ap())
            wz_sb = singles.tile([128, NKC, DSL], bf16)
            nc.sync.dma_start(out=wz_sb, in_=wz.ap())
            wcd_sb = singles.tile([128, NDB, K, 128], bf16)
            nc.sync.dma_start(out=wcd_sb, in_=wcd.ap())
            wxp_sb = singles.tile([128, NDB, NR], bf16)
            nc.sync.dma_start(out=wxp_sb, in_=wxp.ap())
            wdt_sb = singles.tile([R, DSL], bf16)
            nc.sync.dma_start(out=wdt_sb, in_=wdt.ap())
            wo_sb = singles.tile([128, NDB, DM], bf16)
            nc.sync.dma_start(out=wo_sb, in_=wo.ap())
            dtb_sb = singles.tile([128, NDB, 1], f32)
            nc.sync.dma_start(out=dtb_sb, in_=dtb.ap())
            A_sb = singles.tile([128, NDB, S], f32)
            nc.sync.dma_start(out=A_sb, in_=Aneg.ap())
            Dp_sb = singles.tile([128, NDB, 1], f32)
            nc.sync.dma_start(out=Dp_sb, in_=Dp.ap())
            cb_sb = singles.tile([128, NDB, 1], f32)
            nc.sync.dma_start(out=cb_sb, in_=cb.ap())
            cbn_sb = singles.tile([128, NDB, 1], f32)
            nc.vector.tensor_scalar_mul(cbn_sb, cb_sb, -1.0)
            id_sb = singles.tile([128, 128], bf16)
            nc.sync.dma_start(out=id_sb, in_=ident.ap())

            # persistent activations
            u_sb = singles.tile([128, NDB, B, 3 + L], bf16)  # conv input, 3-pad
            uc_sb = singles.tile([128, NDB, T], bf16)        # silu(conv(u))
            sz_sb = singles.tile([128, NDB, T], bf16)        # silu(z)
            dtT_sb = singles.tile([R, T], bf16)              # reduced dt^T
            carry = [[singles.tile([128, SPG], f32, tag=f"carry{db}_{sg}",
                                   name=f"carry{db}_{sg}")
                      for sg in range(SG)] for db in range(NDB)]

            for db in range(NDB):
                for b in range(B):
                    nc.vector.memset(u_sb[:, db, b, 0:3], 0.0)

            # ------- phases 1-4 fused per (b, tc): in_proj u, conv+silu,
            # ------- x_proj partial, allreduce, reduce+broadcast staging ----
            bcst = dram.tile([2 * S, T], bf16)       # broadcast staging
            for b in range(B):
                for tcn in range(NTC):
                    t0 = b * L + tcn * TC
                    xss = []
                    for kc in range(NKC):
                        xs = xs_pool.tile([128, TC], bf16, tag="xs")
                        nc.sync.dma_start(
                            out=xs, in_=xT.ap()[kc * 128:(kc + 1) * 128, t0:t0 + TC])
                        xss.append(xs)
                    for db in range(NDB):
                        pu = psum.tile([128, TC], f32, tag="mm")
                        for kc in range(NKC):
                            nc.tensor.matmul(
                                pu, wu_sb[:, kc, db * 128:(db + 1) * 128], xss[kc],
                                start=(kc == 0), stop=(kc == NKC - 1))
                        nc.scalar.copy(
                            u_sb[:, db, b, 3 + tcn * TC: 3 + (tcn + 1) * TC], pu)
                    for db in range(NDB):
                        pc = psum.tile([128, TC], f32, tag="mm")
                        for k in range(K):
                            nc.tensor.matmul(
                                pc, wcd_sb[:, db, k, :],
                                u_sb[:, db, b, tcn * TC + k: tcn * TC + k + TC],
                                start=(k == 0), stop=(k == K - 1))
                        # silu(pc+cb) = x * 1/(1+exp(-x)); only Exp/Ln/Copy/
                        # Identity live in one ACT LUT table, so build it.
                        xsl = actp.tile([128, TC], bf16, tag="xsl")
                        nc.scalar.activation(xsl, pc, AF.Identity,
                                             bias=cb_sb[:, db, :], scale=1.0)
                        esl = actp.tile([128, TC], bf16, tag="esl")
                        nc.scalar.activation(esl, pc, AF.Exp,
                                             bias=cbn_sb[:, db, :], scale=-1.0)
                        dsl = actp.tile([128, TC], bf16, tag="dsl")
                        nc.vector.tensor_scalar_add(dsl, esl, 1.0)
                        rsl = actp.tile([128, TC], bf16, tag="rsl")
                        nc.vector.reciprocal(rsl, dsl)
                        nc.vector.tensor_tensor(uc_sb[:, db, t0:t0 + TC], xsl, rsl, MUL)
                    px = psum.tile([NR, TC], f32, tag="mm")
                    for db in range(NDB):
                        nc.tensor.matmul(px, wxp_sb[:, db, :], uc_sb[:, db, t0:t0 + TC],
                                         start=(db == 0), stop=(db == NDB - 1))
                    xps = redp.tile([NR, TC], f32, tag="xps")
                    nc.scalar.copy(xps, px)
                    cin = dram.tile([NR, TC], f32, tag=f"cin{b}{tcn}",
                                    name=f"cin{b}{tcn}")
                    cout = dram.tile([NR, TC], f32, tag=f"cout{b}{tcn}",
                                     name=f"cout{b}{tcn}")
                    nc.sync.dma_start(out=cin, in_=xps)
                    if sim_mode:
                        nc.sync.dma_start(out=cout, in_=cin)
                    else:
                        nc.gpsimd.collective_compute(
                            "AllReduce", ADD,
                            replica_groups=[[0, 1, 2, 3], [4, 5, 6, 7]],
                            ins=[cin.opt()], outs=[cout.opt()],
                        )
                    red = redp.tile([R, TC], f32, tag="red")
                    redbc = redp.tile([2 * S, TC], f32, tag="redbc")
                    nc.sync.dma_start(out=red, in_=cout[0:R, :])
                    nc.sync.dma_start(out=redbc, in_=cout[R:NR, :])
                    nc.vector.tensor_copy(dtT_sb[:, t0:t0 + TC], red)
                    bcs = redp.tile([2 * S, TC], bf16, tag="bcs")
                    nc.vector.tensor_copy(bcs, redbc)
                    nc.sync.dma_start(out=bcst[:, t0:t0 + TC], in_=bcs)

            # ------------- phases 6-8: heavy per-(b, tc) pipeline -----------
            for b in range(B):
                for tcn in range(NTC):
                    t0 = b * L + tcn * TC
                    # in_proj z + silu for this chunk (fills PE/ACT while
                    # DVE/Pool chew on the scan stage)
                    xss = []
                    for kc in range(NKC):
                        xs = xs_pool.tile([128, TC], bf16, tag="xs")
                        nc.sync.dma_start(
                            out=xs, in_=xT.ap()[kc * 128:(kc + 1) * 128, t0:t0 + TC])
                        xss.append(xs)
                    for db in range(NDB):
                        pz = psum.tile([128, TC], f32, tag="mm")
                        for kc in range(NKC):
                            nc.tensor.matmul(
                                pz, wz_sb[:, kc, db * 128:(db + 1) * 128], xss[kc],
                                start=(kc == 0), stop=(kc == NKC - 1))
                        xsl = actp.tile([128, TC], bf16, tag="xsl")
                        nc.scalar.copy(xsl, pz)
                        esl = actp.tile([128, TC], bf16, tag="esl")
                        nc.scalar.activation(esl, pz, AF.Exp, scale=-1.0)
                        dsl = actp.tile([128, TC], bf16, tag="dsl")
                        nc.gpsimd.tensor_scalar_add(dsl, esl, 1.0)
                        rsl = actp.tile([128, TC], bf16, tag="rsl")
                        nc.vector.reciprocal(rsl, dsl)
                        nc.gpsimd.tensor_tensor(sz_sb[:, db, t0:t0 + TC], xsl, rsl, MUL)
                    # broadcast B/C rows for this chunk (per s-group tiles)
                    bbc = []
                    cbc = []
                    for sg in range(SG):
                        bt = bcpool.tile([128, SPG, TC], bf16, tag="bbc")
                        ct = bcpool.tile([128, SPG, TC], bf16, tag="cbc")
                        for i in range(SPG):
                            s = sg * SPG + i
                            rb = bcst[s:s + 1, t0:t0 + TC]
                            rc = bcst[S + s:S + s + 1, t0:t0 + TC]
                            nc.sync.dma_start(
                                out=bt[:, i, :],
                                in_=bass.AP(tensor=rb.tensor, offset=rb.offset,
                                            ap=[[0, 128], rb.ap[-1]]))
                            nc.sync.dma_start(
                                out=ct[:, i, :],
                                in_=bass.AP(tensor=rc.tensor, offset=rc.offset,
                                            ap=[[0, 128], rc.ap[-1]]))
                        bbc.append(bt)
                        cbc.append(ct)

                    ys = []
                    for db in range(NDB):
                        # dt_proj -> softplus -> delta (bf16)
                        pdt = psum.tile([128, TC], f32, tag="mm")
                        nc.tensor.matmul(pdt, wdt_sb[:, db * 128:(db + 1) * 128],
                                         dtT_sb[:, t0:t0 + TC], start=True, stop=True)
                        # softplus(pdt + dtb) = Ln(1 + Exp(pdt + dtb))
                        e1 = actp.tile([128, TC], f32, tag="e1")
                        nc.scalar.activation(e1, pdt, AF.Exp,
                                             bias=dtb_sb[:, db, :], scale=1.0)
                        delta = small.tile([128, TC], bf16, tag="delta")
                        nc.scalar.activation(delta, e1, AF.Ln, bias=1.0, scale=1.0)
                        # w = delta * uc
                        w = small.tile([128, TC], bf16, tag="w")
                        nc.vector.tensor_tensor(w, delta, uc_sb[:, db, t0:t0 + TC], MUL)

                        py = psumy.tile([128, TC], f32, tag="py")
                        for sg in range(SG):
                            da = big.tile([128, SPG, TC], bf16, tag="da")
                            for i in range(SPG):
                                s = sg * SPG + i
                                nc.scalar.activation(da[:, i, :], delta, AF.Exp,
                                                     scale=A_sb[:, db, s:s + 1])
                            dbu = big.tile([128, SPG, TC], bf16, tag="dbu")
                            if sg == 0:
                                nc.vector.tensor_tensor(dbu, _bcast_free(w, SPG),
                                                        bbc[sg], MUL)
                            else:
                                # quarter the slow gpsimd op so consumers of
                                # early s-slices start sooner
                                for qr in range(0, SPG, 2):
                                    nc.gpsimd.tensor_tensor(
                                        dbu[:, qr:qr + 2, :],
                                        _bcast_free(w, 2),
                                        bbc[sg][:, qr:qr + 2, :], MUL)
                            h = hpool.tile([128, SPG, TC], bf16, tag="h")
                            for i in range(SPG):
                                init = 0.0 if tcn == 0 else carry[db][sg][:, i:i + 1]
                                nc.vector.tensor_tensor_scan(
                                    h[:, i, :], da[:, i, :], dbu[:, i, :], init,
                                    MUL, ADD)
                            if tcn + 1 < NTC:
                                nc.vector.tensor_copy(carry[db][sg], h[:, :, TC - 1])
                            yp = big.tile([128, SPG, TC], bf16, tag="yp")
                            if sg == 0:
                                for qr in range(0, SPG, 2):
                                    nc.gpsimd.tensor_tensor(
                                        yp[:, qr:qr + 2, :], h[:, qr:qr + 2, :],
                                        cbc[sg][:, qr:qr + 2, :], MUL)
                            else:
                                nc.vector.tensor_tensor(yp, h, cbc[sg], MUL)
                            for i in range(SPG):
                                nc.tensor.matmul(py, id_sb, yp[:, i, :],
                                                 start=(sg == 0 and i == 0),
                                                 stop=(sg == SG - 1 and i == SPG - 1))
                        # y1 = uc*Dp + py ; y2 = y1 * silu(z)
                        y1 = small.tile([128, TC], bf16, tag="y1")
                        nc.vector.scalar_tensor_tensor(
                            y1, uc_sb[:, db, t0:t0 + TC], Dp_sb[:, db, :], py,
                            MUL, ADD)
                        y2t = y2pool.tile([128, TC], bf16, tag="y2")
                        nc.vector.tensor_tensor(y2t, y1, sz_sb[:, db, t0:t0 + TC], MUL)
                        ys.append(y2t)

                    # out_proj partial for this chunk
                    for mc in range(NKC):
                        po = psumo.tile([128, TC], f32, tag="po")
                        for db in range(NDB):
                            nc.tensor.matmul(
                                po, wo_sb[:, db, mc * 128:(mc + 1) * 128], ys[db],
                                start=(db == 0), stop=(db == NDB - 1))
                        ot = otpool.tile([128, TC], f32, tag="ot")
                        nc.scalar.copy(ot, po)
                        nc.sync.dma_start(
                            out=opart.ap()[mc * 128:(mc + 1) * 128, t0:t0 + TC],
                            in_=ot)
    nc.compile()
    return nc


def _get_nc():
    global _NC_CACHE
    if _NC_CACHE is None:
        _NC_CACHE = build_nc()
    return _NC_CACHE


def _bf(a):
    return np.ascontiguousarray(a).astype(ml_dtypes.bfloat16)


def kernel(**inputs):
    hs = np.asarray(inputs["hidden_states"], dtype=np.float32)  # (B, L, DM)
    in_w = np.asarray(inputs["in_proj_w"], dtype=np.float32)    # (2E, DM)
    out_w = np.asarray(inputs["out_proj_w"], dtype=np.float32)  # (DM, E)
    ident = np.eye(128, dtype=np.float32)

    in_maps = []
    for c in range(NCORES):
        d = "f" if c < 4 else "r"
        q = c % 4
        sl = slice(q * DSL, (q + 1) * DSL)
        x = hs if d == "f" else hs[:, ::-1, :]
        xTh = np.ascontiguousarray(x.transpose(2, 0, 1).reshape(DM, T))

        cw = np.asarray(inputs[f"conv_w_{d}"], dtype=np.float32)[sl]   # (384, 4)
        wcdh = np.zeros((NDB, K, 128, 128), np.float32)
        for db in range(NDB):
            for k in range(K):
                np.fill_diagonal(wcdh[db, k], cw[db * 128:(db + 1) * 128, k])

        xpw = np.asarray(inputs[f"x_proj_w_{d}"], dtype=np.float32)    # (80, E)
        wxph = np.stack([xpw[:, q * DSL + db * 128: q * DSL + (db + 1) * 128].T
                         for db in range(NDB)])                         # (3,128,80)
        dtw = np.asarray(inputs[f"dt_w_{d}"], dtype=np.float32)[sl]    # (384, 48)
        woh = np.stack([out_w[:, q * DSL + db * 128: q * DSL + (db + 1) * 128].T
                        for db in range(NDB)])                          # (3,128,768)

        def p_major(a):
            # (NDB, 128, ...) -> (128, NDB, ...) contiguous
            return np.ascontiguousarray(np.moveaxis(a, 1, 0))

        in_maps.append({
            "xT": _bf(xTh),
            "wu": _bf(p_major(in_w[sl].T.reshape(NKC, 128, DSL))),
            "wz": _bf(p_major(in_w[E:][sl].T.reshape(NKC, 128, DSL))),
            "wcd": _bf(np.ascontiguousarray(wcdh.transpose(2, 0, 1, 3))),
            "wxp": _bf(p_major(wxph)),
            "wdt": _bf(dtw.T),
            "wo": _bf(p_major(woh)),
            "dtb": p_major(
                np.asarray(inputs[f"dt_b_{d}"], np.float32)[sl].reshape(NDB, 128, 1)),
            "Aneg": p_major(
                (-np.exp(np.asarray(inputs[f"A_log_{d}"], np.float32)[sl]))
                .reshape(NDB, 128, S)),
            "Dp": p_major(
                np.asarray(inputs[f"D_{d}"], np.float32)[sl].reshape(NDB, 128, 1)),
            "cb": p_major(
                np.asarray(inputs[f"conv_b_{d}"], np.float32)[sl].reshape(NDB, 128, 1)),
            "ident": _bf(ident),
        })

    nc = _get_nc()
    res = run_bass_kernel_spmd(nc, in_maps, core_ids=list(range(NCORES)))

    acc_f = np.zeros((DM, T), np.float32)
    acc_r = np.zeros((DM, T), np.float32)
    for c in range(NCORES):
        if c < 4:
            acc_f += res.results[c]["opart"]
        else:
            acc_r += res.results[c]["opart"]
    out_f = acc_f.reshape(DM, B, L).transpose(1, 2, 0)
    out_r = acc_r.reshape(DM, B, L).transpose(1, 2, 0)[:, ::-1, :]
    return np.ascontiguousarray(out_f + out_r, dtype=np.float32)


# revision 44
# speedup vs baseline: 145.1942x; 1.0554x over previous
"""BiMamba (bidirectional Mamba-1 block) Trainium2 kernel.

Problem: B=2, L=1024, d_model=768, d_inner=1536, d_state=16, dt_rank=48,
d_conv=4; two directions (fwd on x, rev on flip(x)) sharing in/out
projections, outputs added.

Sharding over 8 NeuronCores: core = (direction, quarter of d_inner).
Cores 0-3 forward, 4-7 reverse (host flips x along L for them and unflips
their partial outputs).  Each core owns 384 d_inner channels = 3 partition
blocks of 128.

On-device layout: channels on partitions, time on the free dimension
(t = b*1024 + l, batches concatenated).  Per core:
  - in_proj (u and z slices), depthwise conv (4 accumulating diagonal
    matmuls), x_proj partial, dt_proj, out_proj partial: TensorE, bf16.
  - partial x_proj outputs ([80, 1024] per batch) AllReduce'd across the
    4 cores of the same direction.
  - softplus / exp(A*delta) (per-partition scale) / silu: ScalarE.
  - selective scan: chained tensor_tensor_scan on VectorE (scan opcode
    does not exist on GpSimd); dBu / y*C bf16 tensor_tensor passes split
    between VectorE and GpSimd.
  - sum over d_state: 16 identity matmuls accumulating in PSUM (TensorE).
  - out_proj partials copied PSUM->SBUF (ScalarE) and DMA'd out; the host
    sums the 8 partial (768, 2048) fp32 outputs and undoes transpose/flip.

TimelineSim (cost model, collective stubbed): ~310 us/core end-to-end;
engine busy: Pool 228 us, ACT 216 us, DVE 207 us, PE 137 us.
Measured end-to-end relative L2 error vs the fp32 reference: 6.9e-3.
"""
import sys

sys.path.insert(0, "/opt/trn_rl_repo")

import numpy as np
import ml_dtypes

import concourse.bass as bass
import concourse.bacc as bacc
import concourse.mybir as mybir
import concourse.tile as tile
from concourse.bass_utils import run_bass_kernel_spmd

# ---- problem constants ----
B, L, DM, E, S, R, K = 2, 1024, 768, 1536, 16, 48, 4
NCORES = 8
Q = 4                      # d_inner quarters per direction
DSL = E // Q               # 384 channels per core
NDB = DSL // 128           # 3 partition blocks
T = B * L                  # 2048 tokens, b-major
TC = 512                   # time chunk
NTC = L // TC              # chunks per batch
SG = 2                     # d_state groups
SPG = S // SG              # 8 states per group
NR = R + 2 * S             # 80
NKC = DM // 128            # 6 k-chunks of d_model

bf16 = mybir.dt.bfloat16
f32 = mybir.dt.float32
MUL = mybir.AluOpType.mult
ADD = mybir.AluOpType.add
AF = mybir.ActivationFunctionType

_NC_CACHE = None


def _bcast_free(ap_row, n):
    """Broadcast an AP of shape [p, F] to [p, n, F] with step-0 middle dim."""
    return bass.AP(tensor=ap_row.tensor, offset=ap_row.offset,
                   ap=[ap_row.ap[0], [0, n], ap_row.ap[1]])


def build_nc(sim_mode=False):
    """sim_mode=True: single-core, collective replaced by a DRAM->DRAM DMA
    (same dataflow deps) so TimelineSim can run the kernel."""
    nc = bacc.Bacc("TRN2", target_bir_lowering=False, debug=False,
                   num_devices=1 if sim_mode else NCORES)

    # ---------------- I/O ----------------
    xT = nc.dram_tensor("xT", [DM, T], bf16, kind="ExternalInput")
    wu = nc.dram_tensor("wu", [128, NKC, DSL], bf16, kind="ExternalInput")
    wz = nc.dram_tensor("wz", [128, NKC, DSL], bf16, kind="ExternalInput")
    wcd = nc.dram_tensor("wcd", [128, NDB, K, 128], bf16, kind="ExternalInput")
    wxp = nc.dram_tensor("wxp", [128, NDB, NR], bf16, kind="ExternalInput")
    wdt = nc.dram_tensor("wdt", [R, DSL], bf16, kind="ExternalInput")
    wo = nc.dram_tensor("wo", [128, NDB, DM], bf16, kind="ExternalInput")
    dtb = nc.dram_tensor("dtb", [128, NDB, 1], f32, kind="ExternalInput")
    Aneg = nc.dram_tensor("Aneg", [128, NDB, S], f32, kind="ExternalInput")
    Dp = nc.dram_tensor("Dp", [128, NDB, 1], f32, kind="ExternalInput")
    cb = nc.dram_tensor("cb", [128, NDB, 1], f32, kind="ExternalInput")
    ident = nc.dram_tensor("ident", [128, 128], bf16, kind="ExternalInput")
    opart = nc.dram_tensor("opart", [DM, T], f32, kind="ExternalOutput")

    with tile.TileContext(nc) as tc:
        import contextlib
        ctx = contextlib.ExitStack()
        with ctx:
            ctx.enter_context(nc.allow_low_precision(reason="deliberate bf16 pipeline"))
            singles = ctx.enter_context(tc.tile_pool(name="singles", bufs=1))
            xs_pool = ctx.enter_context(tc.tile_pool(name="xs", bufs=8))
            big = ctx.enter_context(tc.tile_pool(name="big", bufs=2))
            hpool = ctx.enter_context(tc.tile_pool(name="hp", bufs=3))
            bcpool = ctx.enter_context(tc.tile_pool(name="bc", bufs=2))
            small = ctx.enter_context(tc.tile_pool(name="small", bufs=3))
            actp = ctx.enter_context(tc.tile_pool(name="actp", bufs=2))
            redp = ctx.enter_context(tc.tile_pool(name="redp", bufs=1))
            y2pool = ctx.enter_context(tc.tile_pool(name="y2", bufs=4))
            otpool = ctx.enter_context(tc.tile_pool(name="otp", bufs=2))
            psum = ctx.enter_context(tc.tile_pool(name="psum", bufs=3, space="PSUM"))
            psumy = ctx.enter_context(tc.tile_pool(name="psumy", bufs=3, space="PSUM"))
            psumo = ctx.enter_context(tc.tile_pool(name="psumo", bufs=2, space="PSUM"))
            dram = ctx.enter_context(tc.tile_pool(name="dram", bufs=1, space="DRAM"))

            # ------------- persistent weights -------------
            wu_sb = singles.tile([128, NKC, DSL], bf16)   # [k, kc, m]
            nc.sync.dma_start(out=wu_sb, in_=wu.ap())
            wz_sb = singles.tile([128, NKC, DSL], bf16)
            nc.sync.dma_start(out=wz_sb, in_=wz.ap())
            wcd_sb = singles.tile([128, NDB, K, 128], bf16)
            nc.sync.dma_start(out=wcd_sb, in_=wcd.ap())
            wxp_sb = singles.tile([128, NDB, NR], bf16)
            nc.sync.dma_start(out=wxp_sb, in_=wxp.ap())
            wdt_sb = singles.tile([R, DSL], bf16)
            nc.sync.dma_start(out=wdt_sb, in_=wdt.ap())
            wo_sb = singles.tile([128, NDB, DM], bf16)
            nc.sync.dma_start(out=wo_sb, in_=wo.ap())
            dtb_sb = singles.tile([128, NDB, 1], f32)
            nc.sync.dma_start(out=dtb_sb, in_=dtb.ap())
            A_sb = singles.tile([128, NDB, S], f32)
            nc.sync.dma_start(out=A_sb, in_=Aneg.ap())
            Dp_sb = singles.tile([128, NDB, 1], f32)
            nc.sync.dma_start(out=Dp_sb, in_=Dp.ap())
            cb_sb = singles.tile([128, NDB, 1], f32)
            nc.sync.dma_start(out=cb_sb, in_=cb.ap())
            cbn_sb = singles.tile([128, NDB, 1], f32)
            nc.vector.tensor_scalar_mul(cbn_sb, cb_sb, -1.0)
            id_sb = singles.tile([128, 128], bf16)
            nc.sync.dma_start(out=id_sb, in_=ident.ap())

            # persistent activations
            u_sb = singles.tile([128, NDB, B, 3 + L], bf16)  # conv input, 3-pad
            uc_sb = singles.tile([128, NDB, T], bf16)        # silu(conv(u))
            sz_sb = singles.tile([128, NDB, T], bf16)        # silu(z)
            dtT_sb = singles.tile([R, T], bf16)              # reduced dt^T
            carry = [[singles.tile([128, SPG], f32, tag=f"carry{db}_{sg}",
                                   name=f"carry{db}_{sg}")
                      for sg in range(SG)] for db in range(NDB)]

            for db in range(NDB):
                for b in range(B):
                    nc.vector.memset(u_sb[:, db, b, 0:3], 0.0)

            # ------- phases 1-4 fused per (b, tc): in_proj u, conv+silu,
            # ------- x_proj partial, allreduce, reduce+broadcast staging ----
            bcst = dram.tile([2 * S, T], bf16)       # broadcast staging
            for b in range(B):
                for tcn in range(NTC):
                    t0 = b * L + tcn * TC
                    xss = []
                    for kc in range(NKC):
                        xs = xs_pool.tile([128, TC], bf16, tag="xs")
                        nc.sync.dma_start(
                            out=xs, in_=xT.ap()[kc * 128:(kc + 1) * 128, t0:t0 + TC])
                        xss.append(xs)
                    for db in range(NDB):
                        pu = psum.tile([128, TC], f32, tag="mm")
                        for kc in range(NKC):
                            nc.tensor.matmul(
                                pu, wu_sb[:, kc, db * 128:(db + 1) * 128], xss[kc],
                                start=(kc == 0), stop=(kc == NKC - 1))
                        nc.scalar.copy(
                            u_sb[:, db, b, 3 + tcn * TC: 3 + (tcn + 1) * TC], pu)
                    for db in range(NDB):
                        pc = psum.tile([128, TC], f32, tag="mm")
                        for k in range(K):
                            nc.tensor.matmul(
                                pc, wcd_sb[:, db, k, :],
                                u_sb[:, db, b, tcn * TC + k: tcn * TC + k + TC],
                                start=(k == 0), stop=(k == K - 1))
                        # silu(pc+cb) = x * 1/(1+exp(-x)); only Exp/Ln/Copy/
                        # Identity live in one ACT LUT table, so build it.
                        xsl = actp.tile([128, TC], bf16, tag="xsl")
                        nc.scalar.activation(xsl, pc, AF.Identity,
                                             bias=cb_sb[:, db, :], scale=1.0)
                        esl = actp.tile([128, TC], bf16, tag="esl")
                        nc.scalar.activation(esl, pc, AF.Exp,
                                             bias=cbn_sb[:, db, :], scale=-1.0)
                        dsl = actp.tile([128, TC], bf16, tag="dsl")
                        nc.vector.tensor_scalar_add(dsl, esl, 1.0)
                        rsl = actp.tile([128, TC], bf16, tag="rsl")
                        nc.vector.reciprocal(rsl, dsl)
                        nc.vector.tensor_tensor(uc_sb[:, db, t0:t0 + TC], xsl, rsl, MUL)
                    px = psum.tile([NR, TC], f32, tag="mm")
                    for db in range(NDB):
                        nc.tensor.matmul(px, wxp_sb[:, db, :], uc_sb[:, db, t0:t0 + TC],
                                         start=(db == 0), stop=(db == NDB - 1))
                    xps = redp.tile([NR, TC], f32, tag="xps")
                    nc.scalar.copy(xps, px)
                    cin = dram.tile([NR, TC], f32, tag=f"cin{b}{tcn}",
                                    name=f"cin{b}{tcn}")
                    cout = dram.tile([NR, TC], f32, tag=f"cout{b}{tcn}",
                                     name=f"cout{b}{tcn}")
                    nc.sync.dma_start(out=cin, in_=xps)
                    if sim_mode:
                        nc.sync.dma_start(out=cout, in_=cin)
                    else:
                        nc.gpsimd.collective_compute(
                            "AllReduce", ADD,
                            replica_groups=[[0, 1, 2, 3], [4, 5, 6, 7]],
                            ins=[cin.opt()], outs=[cout.opt()],
                        )
                    red = redp.tile([R, TC], f32, tag="red")
                    redbc = redp.tile([2 * S, TC], f32, tag="redbc")
                    nc.sync.dma_start(out=red, in_=cout[0:R, :])
                    nc.sync.dma_start(out=redbc, in_=cout[R:NR, :])
                    nc.vector.tensor_copy(dtT_sb[:, t0:t0 + TC], red)
                    bcs = redp.tile([2 * S, TC], bf16, tag="bcs")
                    nc.vector.tensor_copy(bcs, redbc)
                    nc.sync.dma_start(out=bcst[:, t0:t0 + TC], in_=bcs)

            # ------------- phases 6-8: heavy per-(b, tc) pipeline -----------
            for b in range(B):
                for tcn in range(NTC):
                    t0 = b * L + tcn * TC
                    # in_proj z + silu for this chunk (fills PE/ACT while
                    # DVE/Pool chew on the scan stage)
                    xss = []
                    for kc in range(NKC):
                        xs = xs_pool.tile([128, TC], bf16, tag="xs")
                        nc.sync.dma_start(
                            out=xs, in_=xT.ap()[kc * 128:(kc + 1) * 128, t0:t0 + TC])
                        xss.append(xs)
                    for db in range(NDB):
                        pz = psum.tile([128, TC], f32, tag="mm")
                        for kc in range(NKC):
                            nc.tensor.matmul(
                                pz, wz_sb[:, kc, db * 128:(db + 1) * 128], xss[kc],
                                start=(kc == 0), stop=(kc == NKC - 1))
                        xsl = actp.tile([128, TC], bf16, tag="xsl")
                        nc.scalar.copy(xsl, pz)
                        esl = actp.tile([128, TC], bf16, tag="esl")
                        nc.scalar.activation(esl, pz, AF.Exp, scale=-1.0)
                        dsl = actp.tile([128, TC], bf16, tag="dsl")
                        nc.vector.tensor_scalar_add(dsl, esl, 1.0)
                        rsl = actp.tile([128, TC], bf16, tag="rsl")
                        nc.vector.reciprocal(rsl, dsl)
                        nc.vector.tensor_tensor(sz_sb[:, db, t0:t0 + TC], xsl, rsl, MUL)
                    # broadcast B/C rows for this chunk (per s-group tiles)
                    bbc = []
                    cbc = []
                    for sg in range(SG):
                        bt = bcpool.tile([128, SPG, TC], bf16, tag="bbc")
                        ct = bcpool.tile([128, SPG, TC], bf16, tag="cbc")
                        for i in range(SPG):
                            s = sg * SPG + i
                            rb = bcst[s:s + 1, t0:t0 + TC]
                            rc = bcst[S + s:S + s + 1, t0:t0 + TC]
                            nc.sync.dma_start(
                                out=bt[:, i, :],
                                in_=bass.AP(tensor=rb.tensor, offset=rb.offset,
                                            ap=[[0, 128], rb.ap[-1]]))
                            nc.sync.dma_start(
                                out=ct[:, i, :],
                                in_=bass.AP(tensor=rc.tensor, offset=rc.offset,
                                            ap=[[0, 128], rc.ap[-1]]))
                        bbc.append(bt)
                        cbc.append(ct)

                    ys = []
                    for db in range(NDB):
                        # dt_proj -> softplus -> delta (bf16)
                        pdt = psum.tile([128, TC], f32, tag="mm")
                        nc.tensor.matmul(pdt, wdt_sb[:, db * 128:(db + 1) * 128],
                                         dtT_sb[:, t0:t0 + TC], start=True, stop=True)
                        # softplus(pdt + dtb) = Ln(1 + Exp(pdt + dtb))
                        e1 = actp.tile([128, TC], f32, tag="e1")
                        nc.scalar.activation(e1, pdt, AF.Exp,
                                             bias=dtb_sb[:, db, :], scale=1.0)
                        delta = small.tile([128, TC], bf16, tag="delta")
                        nc.scalar.activation(delta, e1, AF.Ln, bias=1.0, scale=1.0)
                        # w = delta * uc
                        w = small.tile([128, TC], bf16, tag="w")
                        nc.vector.tensor_tensor(w, delta, uc_sb[:, db, t0:t0 + TC], MUL)

                        py = psumy.tile([128, TC], f32, tag="py")
                        for sg in range(SG):
                            da = big.tile([128, SPG, TC], bf16, tag="da")
                            for i in range(SPG):
                                s = sg * SPG + i
                                nc.scalar.activation(da[:, i, :], delta, AF.Exp,
                                                     scale=A_sb[:, db, s:s + 1])
                            dbu = big.tile([128, SPG, TC], bf16, tag="dbu")
                            if sg == 0:
                                nc.vector.tensor_tensor(dbu, _bcast_free(w, SPG),
                                                        bbc[sg], MUL)
                            else:
                                # quarter the slow gpsimd op so consumers of
                                # early s-slices start sooner
                                for qr in range(0, SPG, 2):
                                    nc.gpsimd.tensor_tensor(
                                        dbu[:, qr:qr + 2, :],
                                        _bcast_free(w, 2),
                                        bbc[sg][:, qr:qr + 2, :], MUL)
                            h = hpool.tile([128, SPG, TC], bf16, tag="h")
                            for i in range(SPG):
                                init = 0.0 if tcn == 0 else carry[db][sg][:, i:i + 1]
                                nc.vector.tensor_tensor_scan(
                                    h[:, i, :], da[:, i, :], dbu[:, i, :], init,
                                    MUL, ADD)
                            if tcn + 1 < NTC:
                                nc.vector.tensor_copy(carry[db][sg], h[:, :, TC - 1])
                            yp = big.tile([128, SPG, TC], bf16, tag="yp")
                            if sg == 0:
                                for qr in range(0, SPG, 2):
                                    nc.gpsimd.tensor_tensor(
                                        yp[:, qr:qr + 2, :], h[:, qr:qr + 2, :],
                                        cbc[sg][:, qr:qr + 2, :], MUL)
                            else:
                                nc.vector.tensor_tensor(yp, h, cbc[sg], MUL)
                            for i in range(SPG):
                                nc.tensor.matmul(py, id_sb, yp[:, i, :],
                                                 start=(sg == 0 and i == 0),
                                                 stop=(sg == SG - 1 and i == SPG - 1))
                        # y1 = uc*Dp + py ; y2 = y1 * silu(z)
                        y1 = small.tile([128, TC], bf16, tag="y1")
                        nc.vector.scalar_tensor_tensor(
                            y1, uc_sb[:, db, t0:t0 + TC], Dp_sb[:, db, :], py,
                            MUL, ADD)
                        y2t = y2pool.tile([128, TC], bf16, tag="y2")
                        nc.vector.tensor_tensor(y2t, y1, sz_sb[:, db, t0:t0 + TC], MUL)
                        ys.append(y2t)

                    # out_proj partial for this chunk
                    for mc in range(NKC):
                        po = psumo.tile([128, TC], f32, tag="po")
                        for db in range(NDB):
                            nc.tensor.matmul(
                                po, wo_sb[:, db, mc * 128:(mc + 1) * 128], ys[db],
                                start=(db == 0), stop=(db == NDB - 1))
                        ot = otpool.tile([128, TC], f32, tag="ot")
                        nc.scalar.copy(ot, po)
                        nc.sync.dma_start(
                            out=opart.ap()[mc * 128:(mc + 1) * 128, t0:t0 + TC],
                            in_=ot)
    nc.compile()
    return nc


def _get_nc():
    global _NC_CACHE
    if _NC_CACHE is None:
        _NC_CACHE = build_nc()
    return _NC_CACHE


def _bf(a):
    return np.ascontiguousarray(a).astype(ml_dtypes.bfloat16)


def kernel(**inputs):
    hs = np.asarray(inputs["hidden_states"], dtype=np.float32)  # (B, L, DM)
    in_w = np.asarray(inputs["in_proj_w"], dtype=np.float32)    # (2E, DM)
    out_w = np.asarray(inputs["out_proj_w"], dtype=np.float32)  # (DM, E)
    ident = np.eye(128, dtype=np.float32)

    in_maps = []
    for c in range(NCORES):
        d = "f" if c < 4 else "r"
        q = c % 4
        sl = slice(q * DSL, (q + 1) * DSL)
        x = hs if d == "f" else hs[:, ::-1, :]
        xTh = np.ascontiguousarray(x.transpose(2, 0, 1).reshape(DM, T))

        cw = np.asarray(inputs[f"conv_w_{d}"], dtype=np.float32)[sl]   # (384, 4)
        wcdh = np.zeros((NDB, K, 128, 128), np.float32)
        for db in range(NDB):
            for k in range(K):
                np.fill_diagonal(wcdh[db, k], cw[db * 128:(db + 1) * 128, k])

        xpw = np.asarray(inputs[f"x_proj_w_{d}"], dtype=np.float32)    # (80, E)
        wxph = np.stack([xpw[:, q * DSL + db * 128: q * DSL + (db + 1) * 128].T
                         for db in range(NDB)])                         # (3,128,80)
        dtw = np.asarray(inputs[f"dt_w_{d}"], dtype=np.float32)[sl]    # (384, 48)
        woh = np.stack([out_w[:, q * DSL + db * 128: q * DSL + (db + 1) * 128].T
                        for db in range(NDB)])                          # (3,128,768)

        def p_major(a):
            # (NDB, 128, ...) -> (128, NDB, ...) contiguous
            return np.ascontiguousarray(np.moveaxis(a, 1, 0))

        in_maps.append({
            "xT": _bf(xTh),
            "wu": _bf(p_major(in_w[sl].T.reshape(NKC, 128, DSL))),
            "wz": _bf(p_major(in_w[E:][sl].T.reshape(NKC, 128, DSL))),
            "wcd": _bf(np.ascontiguousarray(wcdh.transpose(2, 0, 1, 3))),
            "wxp": _bf(p_major(wxph)),
            "wdt": _bf(dtw.T),
            "wo": _bf(p_major(woh)),
            "dtb": p_major(
                np.asarray(inputs[f"dt_b_{d}"], np.float32)[sl].reshape(NDB, 128, 1)),
            "Aneg": p_major(
                (-np.exp(np.asarray(inputs[f"A_log_{d}"], np.float32)[sl]))
                .reshape(NDB, 128, S)),
            "Dp": p_major(
                np.asarray(inputs[f"D_{d}"], np.float32)[sl].reshape(NDB, 128, 1)),
            "cb": p_major(
                np.asarray(inputs[f"conv_b_{d}"], np.float32)[sl].reshape(NDB, 128, 1)),
            "ident": _bf(ident),
        })

    nc = _get_nc()
    res = run_bass_kernel_spmd(nc, in_maps, core_ids=list(range(NCORES)))

    acc_f = np.zeros((DM, T), np.float32)
    acc_r = np.zeros((DM, T), np.float32)
    for c in range(NCORES):
        if c < 4:
            acc_f += res.results[c]["opart"]
        else:
            acc_r += res.results[c]["opart"]
    out_f = acc_f.reshape(DM, B, L).transpose(1, 2, 0)
    out_r = acc_r.reshape(DM, B, L).transpose(1, 2, 0)[:, ::-1, :]
    return np.ascontiguousarray(out_f + out_r, dtype=np.float32)


# revision 45
# speedup vs baseline: 145.4754x; 1.0019x over previous
"""BiMamba (bidirectional Mamba-1 block) Trainium2 kernel.

Problem: B=2, L=1024, d_model=768, d_inner=1536, d_state=16, dt_rank=48,
d_conv=4; two directions (fwd on x, rev on flip(x)) sharing in/out
projections, outputs added.

Sharding over 8 NeuronCores: core = (direction, quarter of d_inner).
Cores 0-3 forward, 4-7 reverse (host flips x along L for them and unflips
their partial outputs).  Each core owns 384 d_inner channels = 3 partition
blocks of 128.

On-device layout: channels on partitions, time on the free dimension
(t = b*1024 + l, batches concatenated).  Per core:
  - in_proj (u and z slices), depthwise conv (4 accumulating diagonal
    matmuls), x_proj partial, dt_proj, out_proj partial: TensorE, bf16.
  - partial x_proj outputs ([80, 1024] per batch) AllReduce'd across the
    4 cores of the same direction.
  - softplus / exp(A*delta) (per-partition scale) / silu: ScalarE.
  - selective scan: chained tensor_tensor_scan on VectorE (scan opcode
    does not exist on GpSimd); dBu / y*C bf16 tensor_tensor passes split
    between VectorE and GpSimd.
  - sum over d_state: 16 identity matmuls accumulating in PSUM (TensorE).
  - out_proj partials copied PSUM->SBUF (ScalarE) and DMA'd out; the host
    sums the 8 partial (768, 2048) fp32 outputs and undoes transpose/flip.

TimelineSim (cost model, collective stubbed): ~310 us/core end-to-end;
engine busy: Pool 228 us, ACT 216 us, DVE 207 us, PE 137 us.
Measured end-to-end relative L2 error vs the fp32 reference: 6.9e-3.
"""
import sys

sys.path.insert(0, "/opt/trn_rl_repo")

import numpy as np
import ml_dtypes

import concourse.bass as bass
import concourse.bacc as bacc
import concourse.mybir as mybir
import concourse.tile as tile
from concourse.bass_utils import run_bass_kernel_spmd

# ---- problem constants ----
B, L, DM, E, S, R, K = 2, 1024, 768, 1536, 16, 48, 4
NCORES = 8
Q = 4                      # d_inner quarters per direction
DSL = E // Q               # 384 channels per core
NDB = DSL // 128           # 3 partition blocks
T = B * L                  # 2048 tokens, b-major
TC = 512                   # time chunk
NTC = L // TC              # chunks per batch
SG = 2                     # d_state groups
SPG = S // SG              # 8 states per group
NR = R + 2 * S             # 80
NKC = DM // 128            # 6 k-chunks of d_model

bf16 = mybir.dt.bfloat16
f32 = mybir.dt.float32
MUL = mybir.AluOpType.mult
ADD = mybir.AluOpType.add
AF = mybir.ActivationFunctionType

_NC_CACHE = None


def _bcast_free(ap_row, n):
    """Broadcast an AP of shape [p, F] to [p, n, F] with step-0 middle dim."""
    return bass.AP(tensor=ap_row.tensor, offset=ap_row.offset,
                   ap=[ap_row.ap[0], [0, n], ap_row.ap[1]])


def build_nc(sim_mode=False):
    """sim_mode=True: single-core, collective replaced by a DRAM->DRAM DMA
    (same dataflow deps) so TimelineSim can run the kernel."""
    nc = bacc.Bacc("TRN2", target_bir_lowering=False, debug=False,
                   num_devices=1 if sim_mode else NCORES)

    # ---------------- I/O ----------------
    xT = nc.dram_tensor("xT", [DM, T], bf16, kind="ExternalInput")
    wu = nc.dram_tensor("wu", [128, NKC, DSL], bf16, kind="ExternalInput")
    wz = nc.dram_tensor("wz", [128, NKC, DSL], bf16, kind="ExternalInput")
    wcd = nc.dram_tensor("wcd", [128, NDB, K, 128], bf16, kind="ExternalInput")
    wxp = nc.dram_tensor("wxp", [128, NDB, NR], bf16, kind="ExternalInput")
    wdt = nc.dram_tensor("wdt", [R, DSL], bf16, kind="ExternalInput")
    wo = nc.dram_tensor("wo", [128, NDB, DM], bf16, kind="ExternalInput")
    dtb = nc.dram_tensor("dtb", [128, NDB, 1], f32, kind="ExternalInput")
    Aneg = nc.dram_tensor("Aneg", [128, NDB, S], f32, kind="ExternalInput")
    cb = nc.dram_tensor("cb", [128, NDB, 1], f32, kind="ExternalInput")
    wdp = nc.dram_tensor("wdp", [128, NDB, 128], bf16, kind="ExternalInput")
    ident = nc.dram_tensor("ident", [128, 128], bf16, kind="ExternalInput")
    opart = nc.dram_tensor("opart", [DM, T], f32, kind="ExternalOutput")

    with tile.TileContext(nc) as tc:
        import contextlib
        ctx = contextlib.ExitStack()
        with ctx:
            ctx.enter_context(nc.allow_low_precision(reason="deliberate bf16 pipeline"))
            singles = ctx.enter_context(tc.tile_pool(name="singles", bufs=1))
            xs_pool = ctx.enter_context(tc.tile_pool(name="xs", bufs=8))
            big = ctx.enter_context(tc.tile_pool(name="big", bufs=2))
            hpool = ctx.enter_context(tc.tile_pool(name="hp", bufs=3))
            bcpool = ctx.enter_context(tc.tile_pool(name="bc", bufs=2))
            small = ctx.enter_context(tc.tile_pool(name="small", bufs=3))
            actp = ctx.enter_context(tc.tile_pool(name="actp", bufs=2))
            redp = ctx.enter_context(tc.tile_pool(name="redp", bufs=1))
            y2pool = ctx.enter_context(tc.tile_pool(name="y2", bufs=4))
            otpool = ctx.enter_context(tc.tile_pool(name="otp", bufs=2))
            psum = ctx.enter_context(tc.tile_pool(name="psum", bufs=3, space="PSUM"))
            psumy = ctx.enter_context(tc.tile_pool(name="psumy", bufs=3, space="PSUM"))
            psumo = ctx.enter_context(tc.tile_pool(name="psumo", bufs=2, space="PSUM"))
            dram = ctx.enter_context(tc.tile_pool(name="dram", bufs=1, space="DRAM"))

            # ------------- persistent weights -------------
            wu_sb = singles.tile([128, NKC, DSL], bf16)   # [k, kc, m]
            nc.sync.dma_start(out=wu_sb, in_=wu.ap())
            wz_sb = singles.tile([128, NKC, DSL], bf16)
            nc.sync.dma_start(out=wz_sb, in_=wz.ap())
            wcd_sb = singles.tile([128, NDB, K, 128], bf16)
            nc.sync.dma_start(out=wcd_sb, in_=wcd.ap())
            wxp_sb = singles.tile([128, NDB, NR], bf16)
            nc.sync.dma_start(out=wxp_sb, in_=wxp.ap())
            wdt_sb = singles.tile([R, DSL], bf16)
            nc.sync.dma_start(out=wdt_sb, in_=wdt.ap())
            wo_sb = singles.tile([128, NDB, DM], bf16)
            nc.sync.dma_start(out=wo_sb, in_=wo.ap())
            dtb_sb = singles.tile([128, NDB, 1], f32)
            nc.sync.dma_start(out=dtb_sb, in_=dtb.ap())
            A_sb = singles.tile([128, NDB, S], f32)
            nc.sync.dma_start(out=A_sb, in_=Aneg.ap())
            cb_sb = singles.tile([128, NDB, 1], f32)
            nc.sync.dma_start(out=cb_sb, in_=cb.ap())
            cbn_sb = singles.tile([128, NDB, 1], f32)
            nc.vector.tensor_scalar_mul(cbn_sb, cb_sb, -1.0)
            id_sb = singles.tile([128, 128], bf16)
            nc.sync.dma_start(out=id_sb, in_=ident.ap())
            wdp_sb = singles.tile([128, NDB, 128], bf16)
            nc.sync.dma_start(out=wdp_sb, in_=wdp.ap())

            # persistent activations
            u_sb = singles.tile([128, NDB, B, 3 + L], bf16)  # conv input, 3-pad
            uc_sb = singles.tile([128, NDB, T], bf16)        # silu(conv(u))
            sz_sb = singles.tile([128, NDB, T], bf16)        # silu(z)
            dtT_sb = singles.tile([R, T], bf16)              # reduced dt^T
            carry = [[singles.tile([128, SPG], f32, tag=f"carry{db}_{sg}",
                                   name=f"carry{db}_{sg}")
                      for sg in range(SG)] for db in range(NDB)]

            for db in range(NDB):
                for b in range(B):
                    nc.vector.memset(u_sb[:, db, b, 0:3], 0.0)

            # ------- phases 1-4 fused per (b, tc): in_proj u, conv+silu,
            # ------- x_proj partial, allreduce, reduce+broadcast staging ----
            bcst = dram.tile([2 * S, T], bf16)       # broadcast staging
            for b in range(B):
                for tcn in range(NTC):
                    t0 = b * L + tcn * TC
                    xss = []
                    for kc in range(NKC):
                        xs = xs_pool.tile([128, TC], bf16, tag="xs")
                        nc.sync.dma_start(
                            out=xs, in_=xT.ap()[kc * 128:(kc + 1) * 128, t0:t0 + TC])
                        xss.append(xs)
                    for db in range(NDB):
                        pu = psum.tile([128, TC], f32, tag="mm")
                        for kc in range(NKC):
                            nc.tensor.matmul(
                                pu, wu_sb[:, kc, db * 128:(db + 1) * 128], xss[kc],
                                start=(kc == 0), stop=(kc == NKC - 1))
                        nc.scalar.copy(
                            u_sb[:, db, b, 3 + tcn * TC: 3 + (tcn + 1) * TC], pu)
                    for db in range(NDB):
                        pc = psum.tile([128, TC], f32, tag="mm")
                        for k in range(K):
                            nc.tensor.matmul(
                                pc, wcd_sb[:, db, k, :],
                                u_sb[:, db, b, tcn * TC + k: tcn * TC + k + TC],
                                start=(k == 0), stop=(k == K - 1))
                        # silu(pc+cb) = x * 1/(1+exp(-x)); only Exp/Ln/Copy/
                        # Identity live in one ACT LUT table, so build it.
                        xsl = actp.tile([128, TC], bf16, tag="xsl")
                        nc.scalar.activation(xsl, pc, AF.Identity,
                                             bias=cb_sb[:, db, :], scale=1.0)
                        esl = actp.tile([128, TC], bf16, tag="esl")
                        nc.scalar.activation(esl, pc, AF.Exp,
                                             bias=cbn_sb[:, db, :], scale=-1.0)
                        dsl = actp.tile([128, TC], bf16, tag="dsl")
                        nc.vector.tensor_scalar_add(dsl, esl, 1.0)
                        rsl = actp.tile([128, TC], bf16, tag="rsl")
                        nc.vector.reciprocal(rsl, dsl)
                        nc.vector.tensor_tensor(uc_sb[:, db, t0:t0 + TC], xsl, rsl, MUL)
                    px = psum.tile([NR, TC], f32, tag="mm")
                    for db in range(NDB):
                        nc.tensor.matmul(px, wxp_sb[:, db, :], uc_sb[:, db, t0:t0 + TC],
                                         start=(db == 0), stop=(db == NDB - 1))
                    xps = redp.tile([NR, TC], f32, tag="xps")
                    nc.scalar.copy(xps, px)
                    cin = dram.tile([NR, TC], f32, tag=f"cin{b}{tcn}",
                                    name=f"cin{b}{tcn}")
                    cout = dram.tile([NR, TC], f32, tag=f"cout{b}{tcn}",
                                     name=f"cout{b}{tcn}")
                    nc.sync.dma_start(out=cin, in_=xps)
                    if sim_mode:
                        nc.sync.dma_start(out=cout, in_=cin)
                    else:
                        nc.gpsimd.collective_compute(
                            "AllReduce", ADD,
                            replica_groups=[[0, 1, 2, 3], [4, 5, 6, 7]],
                            ins=[cin.opt()], outs=[cout.opt()],
                        )
                    red = redp.tile([R, TC], f32, tag="red")
                    redbc = redp.tile([2 * S, TC], f32, tag="redbc")
                    nc.sync.dma_start(out=red, in_=cout[0:R, :])
                    nc.sync.dma_start(out=redbc, in_=cout[R:NR, :])
                    nc.vector.tensor_copy(dtT_sb[:, t0:t0 + TC], red)
                    bcs = redp.tile([2 * S, TC], bf16, tag="bcs")
                    nc.vector.tensor_copy(bcs, redbc)
                    nc.sync.dma_start(out=bcst[:, t0:t0 + TC], in_=bcs)

            # ------------- phases 6-8: heavy per-(b, tc) pipeline -----------
            for b in range(B):
                for tcn in range(NTC):
                    t0 = b * L + tcn * TC
                    # in_proj z + silu for this chunk (fills PE/ACT while
                    # DVE/Pool chew on the scan stage)
                    xss = []
                    for kc in range(NKC):
                        xs = xs_pool.tile([128, TC], bf16, tag="xs")
                        nc.sync.dma_start(
                            out=xs, in_=xT.ap()[kc * 128:(kc + 1) * 128, t0:t0 + TC])
                        xss.append(xs)
                    for db in range(NDB):
                        pz = psum.tile([128, TC], f32, tag="mm")
                        for kc in range(NKC):
                            nc.tensor.matmul(
                                pz, wz_sb[:, kc, db * 128:(db + 1) * 128], xss[kc],
                                start=(kc == 0), stop=(kc == NKC - 1))
                        xsl = actp.tile([128, TC], bf16, tag="xsl")
                        nc.scalar.copy(xsl, pz)
                        esl = actp.tile([128, TC], bf16, tag="esl")
                        nc.scalar.activation(esl, pz, AF.Exp, scale=-1.0)
                        dsl = actp.tile([128, TC], bf16, tag="dsl")
                        nc.vector.tensor_scalar_add(dsl, esl, 1.0)
                        rsl = actp.tile([128, TC], bf16, tag="rsl")
                        nc.vector.reciprocal(rsl, dsl)
                        nc.vector.tensor_tensor(sz_sb[:, db, t0:t0 + TC], xsl, rsl, MUL)
                    # broadcast B/C rows for this chunk (per s-group tiles)
                    bbc = []
                    cbc = []
                    for sg in range(SG):
                        bt = bcpool.tile([128, SPG, TC], bf16, tag="bbc")
                        ct = bcpool.tile([128, SPG, TC], bf16, tag="cbc")
                        for i in range(SPG):
                            s = sg * SPG + i
                            rb = bcst[s:s + 1, t0:t0 + TC]
                            rc = bcst[S + s:S + s + 1, t0:t0 + TC]
                            nc.sync.dma_start(
                                out=bt[:, i, :],
                                in_=bass.AP(tensor=rb.tensor, offset=rb.offset,
                                            ap=[[0, 128], rb.ap[-1]]))
                            nc.sync.dma_start(
                                out=ct[:, i, :],
                                in_=bass.AP(tensor=rc.tensor, offset=rc.offset,
                                            ap=[[0, 128], rc.ap[-1]]))
                        bbc.append(bt)
                        cbc.append(ct)

                    ys = []
                    for db in range(NDB):
                        # dt_proj -> softplus -> delta (bf16)
                        pdt = psum.tile([128, TC], f32, tag="mm")
                        nc.tensor.matmul(pdt, wdt_sb[:, db * 128:(db + 1) * 128],
                                         dtT_sb[:, t0:t0 + TC], start=True, stop=True)
                        # softplus(pdt + dtb) = Ln(1 + Exp(pdt + dtb))
                        e1 = actp.tile([128, TC], f32, tag="e1")
                        nc.scalar.activation(e1, pdt, AF.Exp,
                                             bias=dtb_sb[:, db, :], scale=1.0)
                        delta = small.tile([128, TC], bf16, tag="delta")
                        nc.scalar.activation(delta, e1, AF.Ln, bias=1.0, scale=1.0)
                        # w = delta * uc
                        w = small.tile([128, TC], bf16, tag="w")
                        nc.vector.tensor_tensor(w, delta, uc_sb[:, db, t0:t0 + TC], MUL)

                        py = psumy.tile([128, TC], f32, tag="py")
                        for sg in range(SG):
                            da = big.tile([128, SPG, TC], bf16, tag="da")
                            for i in range(SPG):
                                s = sg * SPG + i
                                nc.scalar.activation(da[:, i, :], delta, AF.Exp,
                                                     scale=A_sb[:, db, s:s + 1])
                            dbu = big.tile([128, SPG, TC], bf16, tag="dbu")
                            if sg == 0:
                                nc.vector.tensor_tensor(dbu, _bcast_free(w, SPG),
                                                        bbc[sg], MUL)
                            else:
                                # quarter the slow gpsimd op so consumers of
                                # early s-slices start sooner
                                for qr in range(0, SPG, 2):
                                    nc.gpsimd.tensor_tensor(
                                        dbu[:, qr:qr + 2, :],
                                        _bcast_free(w, 2),
                                        bbc[sg][:, qr:qr + 2, :], MUL)
                            h = hpool.tile([128, SPG, TC], bf16, tag="h")
                            for i in range(SPG):
                                init = 0.0 if tcn == 0 else carry[db][sg][:, i:i + 1]
                                nc.vector.tensor_tensor_scan(
                                    h[:, i, :], da[:, i, :], dbu[:, i, :], init,
                                    MUL, ADD)
                            if tcn + 1 < NTC:
                                nc.vector.tensor_copy(carry[db][sg], h[:, :, TC - 1])
                            yp = big.tile([128, SPG, TC], bf16, tag="yp")
                            if sg == 0:
                                for qr in range(0, SPG, 2):
                                    nc.gpsimd.tensor_tensor(
                                        yp[:, qr:qr + 2, :], h[:, qr:qr + 2, :],
                                        cbc[sg][:, qr:qr + 2, :], MUL)
                            else:
                                nc.vector.tensor_tensor(yp, h, cbc[sg], MUL)
                            for i in range(SPG):
                                nc.tensor.matmul(py, id_sb, yp[:, i, :],
                                                 start=(sg == 0 and i == 0),
                                                 stop=False)
                        # skip connection via PE: py += diag(Dp) @ uc,
                        # then gate straight from PSUM: y2 = py * silu(z)
                        nc.tensor.matmul(py, wdp_sb[:, db, :],
                                         uc_sb[:, db, t0:t0 + TC],
                                         start=False, stop=True)
                        y2t = y2pool.tile([128, TC], bf16, tag="y2")
                        nc.vector.tensor_tensor(y2t, py, sz_sb[:, db, t0:t0 + TC], MUL)
                        ys.append(y2t)

                    # out_proj partial for this chunk
                    for mc in range(NKC):
                        po = psumo.tile([128, TC], f32, tag="po")
                        for db in range(NDB):
                            nc.tensor.matmul(
                                po, wo_sb[:, db, mc * 128:(mc + 1) * 128], ys[db],
                                start=(db == 0), stop=(db == NDB - 1))
                        ot = otpool.tile([128, TC], f32, tag="ot")
                        nc.scalar.copy(ot, po)
                        nc.sync.dma_start(
                            out=opart.ap()[mc * 128:(mc + 1) * 128, t0:t0 + TC],
                            in_=ot)
    nc.compile()
    return nc


def _get_nc():
    global _NC_CACHE
    if _NC_CACHE is None:
        _NC_CACHE = build_nc()
    return _NC_CACHE


def _bf(a):
    return np.ascontiguousarray(a).astype(ml_dtypes.bfloat16)


def kernel(**inputs):
    hs = np.asarray(inputs["hidden_states"], dtype=np.float32)  # (B, L, DM)
    in_w = np.asarray(inputs["in_proj_w"], dtype=np.float32)    # (2E, DM)
    out_w = np.asarray(inputs["out_proj_w"], dtype=np.float32)  # (DM, E)
    ident = np.eye(128, dtype=np.float32)

    in_maps = []
    for c in range(NCORES):
        d = "f" if c < 4 else "r"
        q = c % 4
        sl = slice(q * DSL, (q + 1) * DSL)
        x = hs if d == "f" else hs[:, ::-1, :]
        xTh = np.ascontiguousarray(x.transpose(2, 0, 1).reshape(DM, T))

        cw = np.asarray(inputs[f"conv_w_{d}"], dtype=np.float32)[sl]   # (384, 4)
        Dq = np.asarray(inputs[f"D_{d}"], np.float32)[sl]
        wdph = np.zeros((NDB, 128, 128), np.float32)
        for db in range(NDB):
            np.fill_diagonal(wdph[db], Dq[db * 128:(db + 1) * 128])
        wcdh = np.zeros((NDB, K, 128, 128), np.float32)
        for db in range(NDB):
            for k in range(K):
                np.fill_diagonal(wcdh[db, k], cw[db * 128:(db + 1) * 128, k])

        xpw = np.asarray(inputs[f"x_proj_w_{d}"], dtype=np.float32)    # (80, E)
        wxph = np.stack([xpw[:, q * DSL + db * 128: q * DSL + (db + 1) * 128].T
                         for db in range(NDB)])                         # (3,128,80)
        dtw = np.asarray(inputs[f"dt_w_{d}"], dtype=np.float32)[sl]    # (384, 48)
        woh = np.stack([out_w[:, q * DSL + db * 128: q * DSL + (db + 1) * 128].T
                        for db in range(NDB)])                          # (3,128,768)

        def p_major(a):
            # (NDB, 128, ...) -> (128, NDB, ...) contiguous
            return np.ascontiguousarray(np.moveaxis(a, 1, 0))

        in_maps.append({
            "xT": _bf(xTh),
            "wu": _bf(p_major(in_w[sl].T.reshape(NKC, 128, DSL))),
            "wz": _bf(p_major(in_w[E:][sl].T.reshape(NKC, 128, DSL))),
            "wcd": _bf(np.ascontiguousarray(wcdh.transpose(2, 0, 1, 3))),
            "wxp": _bf(p_major(wxph)),
            "wdt": _bf(dtw.T),
            "wo": _bf(p_major(woh)),
            "dtb": p_major(
                np.asarray(inputs[f"dt_b_{d}"], np.float32)[sl].reshape(NDB, 128, 1)),
            "Aneg": p_major(
                (-np.exp(np.asarray(inputs[f"A_log_{d}"], np.float32)[sl]))
                .reshape(NDB, 128, S)),
            "cb": p_major(
                np.asarray(inputs[f"conv_b_{d}"], np.float32)[sl].reshape(NDB, 128, 1)),
            "wdp": _bf(p_major(wdph)),
            "ident": _bf(ident),
        })

    nc = _get_nc()
    res = run_bass_kernel_spmd(nc, in_maps, core_ids=list(range(NCORES)))

    acc_f = np.zeros((DM, T), np.float32)
    acc_r = np.zeros((DM, T), np.float32)
    for c in range(NCORES):
        if c < 4:
            acc_f += res.results[c]["opart"]
        else:
            acc_r += res.results[c]["opart"]
    out_f = acc_f.reshape(DM, B, L).transpose(1, 2, 0)
    out_r = acc_r.reshape(DM, B, L).transpose(1, 2, 0)[:, ::-1, :]
    return np.ascontiguousarray(out_f + out_r, dtype=np.float32)
